# revision 29
# baseline (speedup 1.0000x reference)
"""Trainium2 Bass kernel for nn_MeshLoss.

The reference loss is:
    loss = mean((network_mesh - fem_mesh)^2)
         + 0.1 * sum_{dx,dy,dz} sum_spatial(mean_{B,C}(diff^2))
The chamfer/KNN block in the reference is dead code (its results are unused),
and `pc` does not influence the output, so the kernel computes only the two
reduction terms.

Sharding (8 cores): pred is viewed as 12*32 = 384 (bc, x) planes of [32, 32];
the 12*31 = 372 planes with x < 31 are regularization bases, 46-47 per core.
On the host each (plane, y<31) pair becomes a 3-row unit [base row, y+1 row,
x+1-plane row]; a core's 48*31 units (zero-padded to 1536) are laid out as
[128, 12, 3, 32], so every difference is an elementwise op over all 128
partitions with the y/z "::-1" bounds expressed as strided access patterns —
no masking, no invalid contributions.  network_mesh/fem_mesh are split into 8
chunks reshaped [128, 384] and packed side by side as [128, 768].

All compute runs on the Vector engine and there are only three DMAs; walrus
in this toolchain rejects instructions with more than 2 sync commands, so the
kernel must stay a single dependency chain (see _fix_drain_waits).
Each core emits per-partition partial sums [128, 4]; the host sums the 8
outputs and applies the 1/N and 0.1/12 weights.
"""

import numpy as np

B, C, X, Y, Z = 4, 3, 32, 32, 32
N_CORES = 8
FEM_TOTAL = B * C * X * Y * Z          # 393216
REG_PLANES = B * C * (X - 1)           # 372 valid base planes
PLANES_PC = 48                         # plane slots per core (8*48 = 384)
UNITS_PC = PLANES_PC * (Y - 1)         # 1488 (plane, y) units per core
KU = 12                                # units per partition (128*12 = 1536)
FEM_P, FEM_F = 128, FEM_TOTAL // N_CORES // 128   # [128, 384] per core

_PROGRAM = None
_HOOK_PATCHED = False
# Bump whenever the BIR post-edit logic changes: the neuron compile cache
# keys on the HLO (which embeds the *unpatched* BIR), so a patch-logic change
# must perturb the program to force a recompile.
_BIR_REV = 11


def _fix_drain_waits(bir_json):
    """Walrus in this toolchain rejects instructions with >2 sync commands;
    Tile's kernel-tail drain waits on every proc used (no transitive
    reduction).  This kernel is a single dependency chain ending in the
    output DMA, whose completion implies every earlier wait, so the drain
    only needs that one semaphore."""
    import json

    j = json.loads(bir_json)
    for f in j.get("functions", []):
        last_dma_update = None
        for bb in f.get("blocks", []):
            for i in bb.get("instructions", []):
                if i.get("opcode") == "DMACopy":
                    ups = (i.get("sync_info") or {}).get("on_update") or []
                    if ups:
                        last_dma_update = ups[-1]
        if last_dma_update is None:
            continue
        for bb in f.get("blocks", []):
            for i in bb.get("instructions", []):
                if i.get("opcode") != "Drain":
                    continue
                si = i.get("sync_info") or {}
                waits = si.get("on_wait") or []
                if len(waits) + len(si.get("on_update") or []) <= 2:
                    continue
                keep = [w for w in waits if w.get("id") == last_dma_update.get("id")]
                assert keep, f"tail drain lacks final-DMA wait: {waits}"
                # Drop even the final-DMA wait: the tail barriers (~1.2us)
                # then overlap the output write's HBM completion latency
                # (~1.9us); the runtime's execute boundary still serializes
                # executions, and the host consumes the output ms later.
                si["on_wait"] = []
    return json.dumps(j).encode()


def _hoist_input_dmas(bir_json, input_names=("ld_a", "ld_b")):
    """Move the input-load DMA triggers to the head of the first block so the
    HBM->SBUF transfers overlap the ~7.5us framework preamble instead of
    starting after it.  The triggers have no waits, their DMAHW semaphore
    updates don't interact with the barrier semaphores, and consumers keep
    their explicit waits, so ordering stays sound."""
    import json

    j = json.loads(bir_json)
    for f in j.get("functions", []):
        blocks = f.get("blocks", [])
        if not blocks:
            continue
        existing = {i.get("name") for bb in blocks for i in bb.get("instructions", [])}
        hoisted = []
        for bb in blocks:
            insts = bb.get("instructions", [])
            keep = []
            for i in insts:
                ins0 = (i.get("ins") or [{}])[0]
                if (i.get("opcode") == "DMACopy"
                        and not (i.get("sync_info") or {}).get("on_wait")
                        and ins0.get("memref") in input_names):
                    hoisted.append(i)
                else:
                    keep.append(i)
            bb["instructions"] = keep
        # Renumber so they sort before the barrier even if the backend orders
        # by instruction id rather than list position.
        for n, i in enumerate(hoisted):
            name = f"I-{n}"
            while name in existing:
                name += "h"
            existing.add(name)
            i["name"] = name
            i["debug"] = 1
        blocks[0]["instructions"] = hoisted + blocks[0]["instructions"]
    return json.dumps(j).encode()


def _strip_entry_barrier(bir_json):
    """Remove the all-engine rendezvous in the first ("main") block.  It only
    serializes engine start-up (~0.5-1us on the slowest engine); the body's
    ordering is fully semaphore-protected, the codegen block-entry sync still
    rendezvouses engines before the body, and the tail barriers handle
    cross-execution semaphore hygiene."""
    import json

    j = json.loads(bir_json)
    for f in j.get("functions", []):
        blocks = f.get("blocks", [])
        if not blocks:
            continue
        b0 = blocks[0]
        b0["instructions"] = [
            i for i in b0.get("instructions", [])
            if i.get("opcode") not in ("Drain", "EventSemaphore")
        ]
        # Also drop the tail's second rendezvous: it follows the semaphore
        # clear (the Pool "ISA" op); engine-stream completion already gates
        # the next execution at the runtime level.
        bl = blocks[-1]
        insts = bl.get("instructions", [])
        clear_idx = max((n for n, i in enumerate(insts)
                         if i.get("opcode") == "ISA"), default=None)
        if clear_idx is not None:
            insts[clear_idx + 1:] = [
                i for i in insts[clear_idx + 1:]
                if i.get("opcode") not in ("Drain", "EventSemaphore")
            ]
    return json.dumps(j).encode()


def _patch_compile_hook():
    global _HOOK_PATCHED
    if _HOOK_PATCHED:
        return
    import concourse.bass2jax as b2j

    orig = b2j.compile_bir_kernel

    def patched(bir_json, tmpdir, neff_name="file.neff"):
        return orig(_hoist_input_dmas(_strip_entry_barrier(
            _fix_drain_waits(bir_json))), tmpdir, neff_name=neff_name)

    b2j.compile_bir_kernel = patched
    _HOOK_PATCHED = True


def _build_program():
    import concourse.bass as bass
    import concourse.mybir as mybir
    from concourse import tile
    from contextlib import ExitStack

    f32 = mybir.dt.float32
    bf16 = mybir.dt.bfloat16
    SUB = mybir.AluOpType.subtract
    MULT = mybir.AluOpType.mult

    KH = KU // 2
    FH = FEM_F // 2
    UW = KH * 3 * Z                      # u columns per partition (576)
    LW = UW + 2 * FH                     # total load width per ring (960)
    nc = bass.Bass()
    nc.dram_tensor(f"patchrev{_BIR_REV}", [1, 1], f32)
    # One contiguous bf16 load per HWDGE ring (ACT carries ld_a, SP ld_b),
    # each [128, 960]: u half (576 cols) ++ net half (192) ++ fem half (192).
    ld_a = nc.declare_dram_parameter("ld_a", [128, LW], bf16, isOutput=False)
    ld_b = nc.declare_dram_parameter("ld_b", [128, LW], bf16, isOutput=False)
    out = nc.declare_dram_parameter("out", [128, 8], f32, isOutput=True)

    with tile.TileContext(nc) as tc, ExitStack() as ctx:
        pool = ctx.enter_context(tc.tile_pool(name="main", bufs=1))

        t_la = pool.tile([128, LW], bf16)
        t_lb = pool.tile([128, LW], bf16)
        nc.scalar.dma_start(out=t_la[:], in_=ld_a[:, :])
        nc.sync.dma_start(out=t_lb[:], in_=ld_b[:, :])

        # cols 1-3 = reg half a dx/dy/dz; cols 4-6 = half b; cols 0,7 = fem
        t_pack = pool.tile([128, 8], f32)

        # regularization partials: unit row 0 = base, 1 = y+1 row, 2 = x+1 row
        for t_l, c0 in ((t_la, 1), (t_lb, 4)):
            t_u = t_l[:, 0:UW].rearrange("p (k r z) -> p k r z", k=KH, r=3)
            base = t_u[:, :, 0, 0:31]
            srcs = [
                (t_u[:, :, 2, 0:31], c0 + 0),   # dx
                (t_u[:, :, 1, 0:31], c0 + 1),   # dy
                (t_u[:, :, 0, 1:32], c0 + 2),   # dz
            ]
            for shifted, col in srcs:
                t_d = pool.tile([128, KH, 31], bf16, tag=f"d{col}")
                t_sq = pool.tile([128, KH, 31], bf16, tag=f"sq{col}")
                nc.vector.tensor_tensor(out=t_d[:], in0=shifted, in1=base, op=SUB)
                nc.vector.scalar_tensor_tensor(
                    out=t_sq[:], in0=t_d[:], scalar=1.0, in1=t_d[:],
                    op0=MULT, op1=MULT,
                    accum_out=t_pack[0:128, col:col + 1])

        # fem MSE partials: (net - fem)^2 row sums -> pack cols 0 and 7
        for t_l, col in ((t_la, 0), (t_lb, 7)):
            t_fd = pool.tile([FEM_P, FH], bf16, tag=f"fd{col}")
            t_fsq = pool.tile([FEM_P, FH], bf16, tag=f"fsq{col}")
            nc.vector.tensor_tensor(out=t_fd[:], in0=t_l[:, UW:UW + FH],
                                    in1=t_l[:, UW + FH:LW], op=SUB)
            nc.vector.scalar_tensor_tensor(out=t_fsq[:], in0=t_fd[:], scalar=1.0,
                                           in1=t_fd[:], op0=MULT, op1=MULT,
                                           accum_out=t_pack[0:FEM_P, col:col + 1])

        nc.sync.dma_start(out=out[:, :], in_=t_pack[:])

    return nc


def _shard_inputs(network_mesh, fem_mesh, pred):
    import ml_dtypes
    bf16 = ml_dtypes.bfloat16
    predf = np.asarray(pred, dtype=np.float32).reshape(B * C, X, Y, Z)
    pad = N_CORES * PLANES_PC
    base_p = np.zeros((pad, Y, Z), np.float32)
    nxt_p = np.zeros((pad, Y, Z), np.float32)
    base_p[:REG_PLANES] = predf[:, : X - 1].reshape(REG_PLANES, Y, Z)
    nxt_p[:REG_PLANES] = predf[:, 1:].reshape(REG_PLANES, Y, Z)
    # [384, 31, 3, 32]: per (plane, y): base row, y+1 row, x+1-plane row
    u_all = np.stack(
        [base_p[:, : Y - 1], base_p[:, 1:], nxt_p[:, : Y - 1]], axis=2
    )
    KH = KU // 2
    FH = FEM_F // 2
    UW = KH * 3 * Z
    netf = np.asarray(network_mesh, dtype=np.float32).reshape(N_CORES, FEM_P, FEM_F)
    femf = np.asarray(fem_mesh, dtype=np.float32).reshape(N_CORES, FEM_P, FEM_F)
    maps = []
    for c in range(N_CORES):
        uc = u_all[PLANES_PC * c : PLANES_PC * (c + 1)].reshape(UNITS_PC, 3, Z)
        up = np.zeros((128 * KU, 3, Z), np.float32)
        up[:UNITS_PC] = uc
        up = up.reshape(128, KU, 3 * Z)
        la = np.concatenate(
            [up[:, :KH].reshape(128, UW), netf[c, :, :FH], femf[c, :, :FH]], axis=1)
        lb = np.concatenate(
            [up[:, KH:].reshape(128, UW), netf[c, :, FH:], femf[c, :, FH:]], axis=1)
        maps.append({
            "ld_a": np.ascontiguousarray(la).astype(bf16),
            "ld_b": np.ascontiguousarray(lb).astype(bf16),
        })
    return maps


def run_sharded(network_mesh, fem_mesh, pred, trace=False):
    """Compile+run on 8 cores; returns (loss_scalar, BassKernelResults)."""
    global _PROGRAM
    from concourse.bass_utils import run_bass_kernel_spmd

    _patch_compile_hook()
    if _PROGRAM is None:
        _PROGRAM = _build_program()
    in_maps = _shard_inputs(network_mesh, fem_mesh, pred)
    res = run_bass_kernel_spmd(_PROGRAM, in_maps, list(range(N_CORES)), trace=trace)
    fem_sum = 0.0
    reg_sum = 0.0
    for c in range(N_CORES):
        o = np.asarray(res.results[c]["out"], dtype=np.float64)
        fem_sum += o[:, 0].sum() + o[:, 7].sum()
        reg_sum += o[:, 1:7].sum()
    loss = fem_sum / FEM_TOTAL + 0.1 * (reg_sum / (B * C))
    return np.asarray(loss, dtype=np.float32), res


def kernel(network_mesh, pc, fem_mesh, pred):
    loss, _ = run_sharded(network_mesh, fem_mesh, pred, trace=False)
    return loss


# revision 30
# speedup vs baseline: 1.0343x; 1.0343x over previous
"""Trainium2 Bass kernel for nn_MeshLoss.

The reference loss is:
    loss = mean((network_mesh - fem_mesh)^2)
         + 0.1 * sum_{dx,dy,dz} sum_spatial(mean_{B,C}(diff^2))
The chamfer/KNN block in the reference is dead code (its results are unused),
and `pc` does not influence the output, so the kernel computes only the two
reduction terms.

Sharding (8 cores): pred is viewed as 12*32 = 384 (bc, x) planes of [32, 32];
the 12*31 = 372 planes with x < 31 are regularization bases, 46-47 per core.
On the host each (plane, y<31) pair becomes a 3-row unit [base row, y+1 row,
x+1-plane row]; a core's 48*31 units (zero-padded to 1536) are laid out as
[128, 12, 3, 32], so every difference is an elementwise op over all 128
partitions with the y/z "::-1" bounds expressed as strided access patterns —
no masking, no invalid contributions.  network_mesh/fem_mesh are split into 8
chunks reshaped [128, 384] and packed side by side as [128, 768].

All compute runs on the Vector engine and there are only three DMAs; walrus
in this toolchain rejects instructions with more than 2 sync commands, so the
kernel must stay a single dependency chain (see _fix_drain_waits).
Each core emits per-partition partial sums [128, 4]; the host sums the 8
outputs and applies the 1/N and 0.1/12 weights.
"""

import numpy as np

B, C, X, Y, Z = 4, 3, 32, 32, 32
N_CORES = 8
FEM_TOTAL = B * C * X * Y * Z          # 393216
REG_PLANES = B * C * (X - 1)           # 372 valid base planes
PLANES_PC = 48                         # plane slots per core (8*48 = 384)
UNITS_PC = PLANES_PC * (Y - 1)         # 1488 (plane, y) units per core
KU = 12                                # units per partition (128*12 = 1536)
FEM_P, FEM_F = 128, FEM_TOTAL // N_CORES // 128   # [128, 384] per core

_PROGRAM = None
_HOOK_PATCHED = False
# Bump whenever the BIR post-edit logic changes: the neuron compile cache
# keys on the HLO (which embeds the *unpatched* BIR), so a patch-logic change
# must perturb the program to force a recompile.
_BIR_REV = 12


def _fix_drain_waits(bir_json):
    """Walrus in this toolchain rejects instructions with >2 sync commands;
    Tile's kernel-tail drain waits on every proc used (no transitive
    reduction).  This kernel is a single dependency chain ending in the
    output DMA, whose completion implies every earlier wait, so the drain
    only needs that one semaphore."""
    import json

    j = json.loads(bir_json)
    for f in j.get("functions", []):
        last_dma_update = None
        for bb in f.get("blocks", []):
            for i in bb.get("instructions", []):
                if i.get("opcode") == "DMACopy":
                    ups = (i.get("sync_info") or {}).get("on_update") or []
                    if ups:
                        last_dma_update = ups[-1]
        if last_dma_update is None:
            continue
        for bb in f.get("blocks", []):
            for i in bb.get("instructions", []):
                if i.get("opcode") != "Drain":
                    continue
                si = i.get("sync_info") or {}
                waits = si.get("on_wait") or []
                if len(waits) + len(si.get("on_update") or []) <= 2:
                    continue
                keep = [w for w in waits if w.get("id") == last_dma_update.get("id")]
                assert keep, f"tail drain lacks final-DMA wait: {waits}"
                # Drop even the final-DMA wait: the tail barriers (~1.2us)
                # then overlap the output write's HBM completion latency
                # (~1.9us); the runtime's execute boundary still serializes
                # executions, and the host consumes the output ms later.
                si["on_wait"] = []
    return json.dumps(j).encode()


def _hoist_input_dmas(bir_json, input_names=("ld_a", "ld_b")):
    """Move the input-load DMA triggers to the head of the first block so the
    HBM->SBUF transfers overlap the ~7.5us framework preamble instead of
    starting after it.  The triggers have no waits, their DMAHW semaphore
    updates don't interact with the barrier semaphores, and consumers keep
    their explicit waits, so ordering stays sound."""
    import json

    j = json.loads(bir_json)
    for f in j.get("functions", []):
        blocks = f.get("blocks", [])
        if not blocks:
            continue
        existing = {i.get("name") for bb in blocks for i in bb.get("instructions", [])}
        hoisted = []
        for bb in blocks:
            insts = bb.get("instructions", [])
            keep = []
            for i in insts:
                ins0 = (i.get("ins") or [{}])[0]
                if (i.get("opcode") == "DMACopy"
                        and not (i.get("sync_info") or {}).get("on_wait")
                        and ins0.get("memref") in input_names):
                    hoisted.append(i)
                else:
                    keep.append(i)
            bb["instructions"] = keep
        # Renumber so they sort before the barrier even if the backend orders
        # by instruction id rather than list position.
        for n, i in enumerate(hoisted):
            name = f"I-{n}"
            while name in existing:
                name += "h"
            existing.add(name)
            i["name"] = name
            i["debug"] = 1
        blocks[0]["instructions"] = hoisted + blocks[0]["instructions"]
    return json.dumps(j).encode()


def _strip_entry_barrier(bir_json):
    """Remove the all-engine rendezvous in the first ("main") block.  It only
    serializes engine start-up (~0.5-1us on the slowest engine); the body's
    ordering is fully semaphore-protected, the codegen block-entry sync still
    rendezvouses engines before the body, and the tail barriers handle
    cross-execution semaphore hygiene."""
    import json

    j = json.loads(bir_json)
    for f in j.get("functions", []):
        blocks = f.get("blocks", [])
        if not blocks:
            continue
        b0 = blocks[0]
        b0["instructions"] = [
            i for i in b0.get("instructions", [])
            if i.get("opcode") not in ("Drain", "EventSemaphore")
        ]
        # Also drop the tail's second rendezvous: it follows the semaphore
        # clear (the Pool "ISA" op); engine-stream completion already gates
        # the next execution at the runtime level.
        bl = blocks[-1]
        insts = bl.get("instructions", [])
        clear_idx = max((n for n, i in enumerate(insts)
                         if i.get("opcode") == "ISA"), default=None)
        if clear_idx is not None:
            insts[clear_idx + 1:] = [
                i for i in insts[clear_idx + 1:]
                if i.get("opcode") not in ("Drain", "EventSemaphore")
            ]
    return json.dumps(j).encode()


def _patch_compile_hook():
    global _HOOK_PATCHED
    if _HOOK_PATCHED:
        return
    import concourse.bass2jax as b2j

    orig = b2j.compile_bir_kernel

    def patched(bir_json, tmpdir, neff_name="file.neff"):
        return orig(_hoist_input_dmas(_strip_entry_barrier(
            _fix_drain_waits(bir_json))), tmpdir, neff_name=neff_name)

    b2j.compile_bir_kernel = patched
    _HOOK_PATCHED = True


def _build_program():
    import concourse.bass as bass
    import concourse.mybir as mybir
    from concourse import tile
    from contextlib import ExitStack

    f32 = mybir.dt.float32
    bf16 = mybir.dt.bfloat16
    SUB = mybir.AluOpType.subtract
    MULT = mybir.AluOpType.mult

    KH = KU // 2
    FH = FEM_F // 2
    UW = KH * 3 * Z                      # u columns per partition (576)
    LW = UW + 2 * FH                     # total load width per ring (960)
    nc = bass.Bass()
    nc.dram_tensor(f"patchrev{_BIR_REV}", [1, 1], f32)
    # One contiguous bf16 load per HWDGE ring (ACT carries ld_a, SP ld_b),
    # each [128, 960]: u half (576 cols) ++ net half (192) ++ fem half (192).
    ld_a = nc.declare_dram_parameter("ld_a", [128, LW], bf16, isOutput=False)
    ld_b = nc.declare_dram_parameter("ld_b", [128, LW], bf16, isOutput=False)
    out = nc.declare_dram_parameter("out", [128, 2], f32, isOutput=True)

    with tile.TileContext(nc) as tc, ExitStack() as ctx:
        pool = ctx.enter_context(tc.tile_pool(name="main", bufs=1))

        t_la = pool.tile([128, LW], bf16)
        t_lb = pool.tile([128, LW], bf16)
        nc.scalar.dma_start(out=t_la[:], in_=ld_a[:, :])
        nc.sync.dma_start(out=t_lb[:], in_=ld_b[:, :])

        # All eight differences land in disjoint slices of one tile; then a
        # single fused square+accumulate per weight group: reg -> pack col 1,
        # fem -> pack col 0 (separate columns because the loss weights differ).
        RW = 6 * KH * 31                 # 1116 reg diff columns
        TW = RW + 2 * FH                 # + 384 fem diff columns
        t_pack = pool.tile([128, 2], f32)
        t_dall = pool.tile([128, TW], bf16)

        off = 0
        for t_l in (t_la, t_lb):
            t_u = t_l[:, 0:UW].rearrange("p (k r z) -> p k r z", k=KH, r=3)
            base = t_u[:, :, 0, 0:31]
            for shifted in (t_u[:, :, 2, 0:31],    # dx
                            t_u[:, :, 1, 0:31],    # dy
                            t_u[:, :, 0, 1:32]):   # dz
                dst = t_dall[:, off:off + KH * 31].rearrange(
                    "p (k z) -> p k z", z=31)
                nc.vector.tensor_tensor(out=dst, in0=shifted, in1=base, op=SUB)
                off += KH * 31
        for t_l in (t_la, t_lb):
            nc.vector.tensor_tensor(out=t_dall[:, off:off + FH],
                                    in0=t_l[:, UW:UW + FH],
                                    in1=t_l[:, UW + FH:LW], op=SUB)
            off += FH

        t_sqr = pool.tile([128, RW], bf16)
        t_sqf = pool.tile([128, 2 * FH], bf16)
        nc.vector.scalar_tensor_tensor(
            out=t_sqr[:], in0=t_dall[:, 0:RW], scalar=1.0,
            in1=t_dall[:, 0:RW], op0=MULT, op1=MULT,
            accum_out=t_pack[0:128, 1:2])
        nc.vector.scalar_tensor_tensor(
            out=t_sqf[:], in0=t_dall[:, RW:TW], scalar=1.0,
            in1=t_dall[:, RW:TW], op0=MULT, op1=MULT,
            accum_out=t_pack[0:128, 0:1])

        nc.sync.dma_start(out=out[:, :], in_=t_pack[:])

    return nc


def _shard_inputs(network_mesh, fem_mesh, pred):
    import ml_dtypes
    bf16 = ml_dtypes.bfloat16
    predf = np.asarray(pred, dtype=np.float32).reshape(B * C, X, Y, Z)
    pad = N_CORES * PLANES_PC
    base_p = np.zeros((pad, Y, Z), np.float32)
    nxt_p = np.zeros((pad, Y, Z), np.float32)
    base_p[:REG_PLANES] = predf[:, : X - 1].reshape(REG_PLANES, Y, Z)
    nxt_p[:REG_PLANES] = predf[:, 1:].reshape(REG_PLANES, Y, Z)
    # [384, 31, 3, 32]: per (plane, y): base row, y+1 row, x+1-plane row
    u_all = np.stack(
        [base_p[:, : Y - 1], base_p[:, 1:], nxt_p[:, : Y - 1]], axis=2
    )
    KH = KU // 2
    FH = FEM_F // 2
    UW = KH * 3 * Z
    netf = np.asarray(network_mesh, dtype=np.float32).reshape(N_CORES, FEM_P, FEM_F)
    femf = np.asarray(fem_mesh, dtype=np.float32).reshape(N_CORES, FEM_P, FEM_F)
    maps = []
    for c in range(N_CORES):
        uc = u_all[PLANES_PC * c : PLANES_PC * (c + 1)].reshape(UNITS_PC, 3, Z)
        up = np.zeros((128 * KU, 3, Z), np.float32)
        up[:UNITS_PC] = uc
        up = up.reshape(128, KU, 3 * Z)
        la = np.concatenate(
            [up[:, :KH].reshape(128, UW), netf[c, :, :FH], femf[c, :, :FH]], axis=1)
        lb = np.concatenate(
            [up[:, KH:].reshape(128, UW), netf[c, :, FH:], femf[c, :, FH:]], axis=1)
        maps.append({
            "ld_a": np.ascontiguousarray(la).astype(bf16),
            "ld_b": np.ascontiguousarray(lb).astype(bf16),
        })
    return maps


def run_sharded(network_mesh, fem_mesh, pred, trace=False):
    """Compile+run on 8 cores; returns (loss_scalar, BassKernelResults)."""
    global _PROGRAM
    from concourse.bass_utils import run_bass_kernel_spmd

    _patch_compile_hook()
    if _PROGRAM is None:
        _PROGRAM = _build_program()
    in_maps = _shard_inputs(network_mesh, fem_mesh, pred)
    res = run_bass_kernel_spmd(_PROGRAM, in_maps, list(range(N_CORES)), trace=trace)
    fem_sum = 0.0
    reg_sum = 0.0
    for c in range(N_CORES):
        o = np.asarray(res.results[c]["out"], dtype=np.float64)
        fem_sum += o[:, 0].sum()
        reg_sum += o[:, 1].sum()
    loss = fem_sum / FEM_TOTAL + 0.1 * (reg_sum / (B * C))
    return np.asarray(loss, dtype=np.float32), res


def kernel(network_mesh, pc, fem_mesh, pred):
    loss, _ = run_sharded(network_mesh, fem_mesh, pred, trace=False)
    return loss


# revision 31
# speedup vs baseline: 1.0432x; 1.0086x over previous
"""Trainium2 Bass kernel for nn_MeshLoss.

The reference loss is:
    loss = mean((network_mesh - fem_mesh)^2)
         + 0.1 * sum_{dx,dy,dz} sum_spatial(mean_{B,C}(diff^2))
The chamfer/KNN block in the reference is dead code (its results are unused),
and `pc` does not influence the output, so the kernel computes only the two
reduction terms.

Sharding (8 cores): pred is viewed as 12*32 = 384 (bc, x) planes of [32, 32];
the 12*31 = 372 planes with x < 31 are regularization bases, 46-47 per core.
On the host each (plane, y<31) pair becomes a 3-row unit [base row, y+1 row,
x+1-plane row]; a core's 48*31 units (zero-padded to 1536) span all 128 SBUF
partitions, so every difference is an elementwise op with the y/z "::-1"
bounds expressed as strided access patterns — no masking, no invalid
contributions.  network_mesh/fem_mesh are split into per-core [128, 384]
chunks.  All inputs are shipped as bf16 (the fp32 accumulators keep the
result to ~1e-5 relative) as two contiguous [128, 960] loads per core, one
per HWDGE ring (ACT + SP) so they transfer in parallel and overlap the
framework preamble (the triggers are hoisted to the head of the program).

All compute runs on the Vector engine: 8 bf16 subtracts (2x perf mode) into
one concatenated diff tile, then two fused square+accumulate ops producing
per-partition partial sums [128, 2] (col 0 fem, col 1 reg).  The host sums
the 8 outputs and applies the 1/N and 0.1/12 weights.

This toolchain's walrus rejects instructions with more than 2 sync commands,
so the kernel stays a single dependency chain and the BIR is post-processed
(_fix_drain_waits / _hoist_input_dmas / _strip_entry_barrier) before compile.
"""

import numpy as np

B, C, X, Y, Z = 4, 3, 32, 32, 32
N_CORES = 8
FEM_TOTAL = B * C * X * Y * Z          # 393216
REG_PLANES = B * C * (X - 1)           # 372 valid base planes
PLANES_PC = 48                         # plane slots per core (8*48 = 384)
UNITS_PC = PLANES_PC * (Y - 1)         # 1488 (plane, y) units per core
KU = 12                                # units per partition (128*12 = 1536)
FEM_P, FEM_F = 128, FEM_TOTAL // N_CORES // 128   # [128, 384] per core

_PROGRAM = None
_HOOK_PATCHED = False
# Bump whenever the BIR post-edit logic changes: the neuron compile cache
# keys on the HLO (which embeds the *unpatched* BIR), so a patch-logic change
# must perturb the program to force a recompile.
_BIR_REV = 12


def _fix_drain_waits(bir_json):
    """Walrus in this toolchain rejects instructions with >2 sync commands;
    Tile's kernel-tail drain waits on every proc used (no transitive
    reduction).  This kernel is a single dependency chain ending in the
    output DMA, whose completion implies every earlier wait, so the drain
    only needs that one semaphore."""
    import json

    j = json.loads(bir_json)
    for f in j.get("functions", []):
        last_dma_update = None
        for bb in f.get("blocks", []):
            for i in bb.get("instructions", []):
                if i.get("opcode") == "DMACopy":
                    ups = (i.get("sync_info") or {}).get("on_update") or []
                    if ups:
                        last_dma_update = ups[-1]
        if last_dma_update is None:
            continue
        for bb in f.get("blocks", []):
            for i in bb.get("instructions", []):
                if i.get("opcode") != "Drain":
                    continue
                si = i.get("sync_info") or {}
                waits = si.get("on_wait") or []
                if len(waits) + len(si.get("on_update") or []) <= 2:
                    continue
                keep = [w for w in waits if w.get("id") == last_dma_update.get("id")]
                assert keep, f"tail drain lacks final-DMA wait: {waits}"
                # Drop even the final-DMA wait: the tail barriers (~1.2us)
                # then overlap the output write's HBM completion latency
                # (~1.9us); the runtime's execute boundary still serializes
                # executions, and the host consumes the output ms later.
                si["on_wait"] = []
    return json.dumps(j).encode()


def _hoist_input_dmas(bir_json, input_names=("ld_a", "ld_b")):
    """Move the input-load DMA triggers to the head of the first block so the
    HBM->SBUF transfers overlap the ~7.5us framework preamble instead of
    starting after it.  The triggers have no waits, their DMAHW semaphore
    updates don't interact with the barrier semaphores, and consumers keep
    their explicit waits, so ordering stays sound."""
    import json

    j = json.loads(bir_json)
    for f in j.get("functions", []):
        blocks = f.get("blocks", [])
        if not blocks:
            continue
        existing = {i.get("name") for bb in blocks for i in bb.get("instructions", [])}
        hoisted = []
        for bb in blocks:
            insts = bb.get("instructions", [])
            keep = []
            for i in insts:
                ins0 = (i.get("ins") or [{}])[0]
                if (i.get("opcode") == "DMACopy"
                        and not (i.get("sync_info") or {}).get("on_wait")
                        and ins0.get("memref") in input_names):
                    hoisted.append(i)
                else:
                    keep.append(i)
            bb["instructions"] = keep
        # Renumber so they sort before the barrier even if the backend orders
        # by instruction id rather than list position.
        for n, i in enumerate(hoisted):
            name = f"I-{n}"
            while name in existing:
                name += "h"
            existing.add(name)
            i["name"] = name
            i["debug"] = 1
        blocks[0]["instructions"] = hoisted + blocks[0]["instructions"]
    return json.dumps(j).encode()


def _strip_entry_barrier(bir_json):
    """Remove the all-engine rendezvous in the first ("main") block.  It only
    serializes engine start-up (~0.5-1us on the slowest engine); the body's
    ordering is fully semaphore-protected, the codegen block-entry sync still
    rendezvouses engines before the body, and the tail barriers handle
    cross-execution semaphore hygiene."""
    import json

    j = json.loads(bir_json)
    for f in j.get("functions", []):
        blocks = f.get("blocks", [])
        if not blocks:
            continue
        b0 = blocks[0]
        b0["instructions"] = [
            i for i in b0.get("instructions", [])
            if i.get("opcode") not in ("Drain", "EventSemaphore")
        ]
        # Also drop the tail's second rendezvous: it follows the semaphore
        # clear (the Pool "ISA" op); engine-stream completion already gates
        # the next execution at the runtime level.
        bl = blocks[-1]
        insts = bl.get("instructions", [])
        clear_idx = max((n for n, i in enumerate(insts)
                         if i.get("opcode") == "ISA"), default=None)
        if clear_idx is not None:
            insts[clear_idx + 1:] = [
                i for i in insts[clear_idx + 1:]
                if i.get("opcode") not in ("Drain", "EventSemaphore")
            ]
    return json.dumps(j).encode()


def _patch_compile_hook():
    global _HOOK_PATCHED
    if _HOOK_PATCHED:
        return
    import concourse.bass2jax as b2j

    orig = b2j.compile_bir_kernel

    def patched(bir_json, tmpdir, neff_name="file.neff"):
        return orig(_hoist_input_dmas(_strip_entry_barrier(
            _fix_drain_waits(bir_json))), tmpdir, neff_name=neff_name)

    b2j.compile_bir_kernel = patched
    _HOOK_PATCHED = True


def _build_program():
    import concourse.bass as bass
    import concourse.mybir as mybir
    from concourse import tile
    from contextlib import ExitStack

    f32 = mybir.dt.float32
    bf16 = mybir.dt.bfloat16
    SUB = mybir.AluOpType.subtract
    MULT = mybir.AluOpType.mult

    KH = KU // 2
    FH = FEM_F // 2
    UW = KH * 3 * Z                      # u columns per partition (576)
    LW = UW + 2 * FH                     # total load width per ring (960)
    nc = bass.Bass()
    nc.dram_tensor(f"patchrev{_BIR_REV}", [1, 1], f32)
    # One contiguous bf16 load per HWDGE ring (ACT carries ld_a, SP ld_b),
    # each [128, 960]: u half (576 cols) ++ net half (192) ++ fem half (192).
    ld_a = nc.declare_dram_parameter("ld_a", [128, LW], bf16, isOutput=False)
    ld_b = nc.declare_dram_parameter("ld_b", [128, LW], bf16, isOutput=False)
    out = nc.declare_dram_parameter("out", [128, 2], f32, isOutput=True)

    with tile.TileContext(nc) as tc, ExitStack() as ctx:
        pool = ctx.enter_context(tc.tile_pool(name="main", bufs=1))

        t_la = pool.tile([128, LW], bf16)
        t_lb = pool.tile([128, LW], bf16)
        nc.scalar.dma_start(out=t_la[:], in_=ld_a[:, :])
        nc.sync.dma_start(out=t_lb[:], in_=ld_b[:, :])

        # All eight differences land in disjoint slices of one tile; then a
        # single fused square+accumulate per weight group: reg -> pack col 1,
        # fem -> pack col 0 (separate columns because the loss weights differ).
        RW = 6 * KH * 31                 # 1116 reg diff columns
        TW = RW + 2 * FH                 # + 384 fem diff columns
        t_pack = pool.tile([128, 2], f32)
        t_dall = pool.tile([128, TW], bf16)

        off = 0
        for t_l in (t_la, t_lb):
            t_u = t_l[:, 0:UW].rearrange("p (k r z) -> p k r z", k=KH, r=3)
            base = t_u[:, :, 0, 0:31]
            for shifted in (t_u[:, :, 2, 0:31],    # dx
                            t_u[:, :, 1, 0:31],    # dy
                            t_u[:, :, 0, 1:32]):   # dz
                dst = t_dall[:, off:off + KH * 31].rearrange(
                    "p (k z) -> p k z", z=31)
                nc.vector.tensor_tensor(out=dst, in0=shifted, in1=base, op=SUB)
                off += KH * 31
        for t_l in (t_la, t_lb):
            nc.vector.tensor_tensor(out=t_dall[:, off:off + FH],
                                    in0=t_l[:, UW:UW + FH],
                                    in1=t_l[:, UW + FH:LW], op=SUB)
            off += FH

        t_sqr = pool.tile([128, RW], bf16)
        t_sqf = pool.tile([128, 2 * FH], bf16)
        nc.vector.scalar_tensor_tensor(
            out=t_sqr[:], in0=t_dall[:, 0:RW], scalar=1.0,
            in1=t_dall[:, 0:RW], op0=MULT, op1=MULT,
            accum_out=t_pack[0:128, 1:2])
        nc.vector.scalar_tensor_tensor(
            out=t_sqf[:], in0=t_dall[:, RW:TW], scalar=1.0,
            in1=t_dall[:, RW:TW], op0=MULT, op1=MULT,
            accum_out=t_pack[0:128, 0:1])

        nc.sync.dma_start(out=out[:, :], in_=t_pack[:])

    return nc


def _shard_inputs(network_mesh, fem_mesh, pred):
    import ml_dtypes
    bf16 = ml_dtypes.bfloat16
    predf = np.asarray(pred, dtype=np.float32).reshape(B * C, X, Y, Z)
    pad = N_CORES * PLANES_PC
    base_p = np.zeros((pad, Y, Z), np.float32)
    nxt_p = np.zeros((pad, Y, Z), np.float32)
    base_p[:REG_PLANES] = predf[:, : X - 1].reshape(REG_PLANES, Y, Z)
    nxt_p[:REG_PLANES] = predf[:, 1:].reshape(REG_PLANES, Y, Z)
    # [384, 31, 3, 32]: per (plane, y): base row, y+1 row, x+1-plane row
    u_all = np.stack(
        [base_p[:, : Y - 1], base_p[:, 1:], nxt_p[:, : Y - 1]], axis=2
    )
    KH = KU // 2
    FH = FEM_F // 2
    UW = KH * 3 * Z
    netf = np.asarray(network_mesh, dtype=np.float32).reshape(N_CORES, FEM_P, FEM_F)
    femf = np.asarray(fem_mesh, dtype=np.float32).reshape(N_CORES, FEM_P, FEM_F)
    maps = []
    for c in range(N_CORES):
        uc = u_all[PLANES_PC * c : PLANES_PC * (c + 1)].reshape(UNITS_PC, 3, Z)
        up = np.zeros((128 * KU, 3, Z), np.float32)
        up[:UNITS_PC] = uc
        up = up.reshape(128, KU, 3 * Z)
        la = np.concatenate(
            [up[:, :KH].reshape(128, UW), netf[c, :, :FH], femf[c, :, :FH]], axis=1)
        lb = np.concatenate(
            [up[:, KH:].reshape(128, UW), netf[c, :, FH:], femf[c, :, FH:]], axis=1)
        maps.append({
            "ld_a": np.ascontiguousarray(la).astype(bf16),
            "ld_b": np.ascontiguousarray(lb).astype(bf16),
        })
    return maps


def run_sharded(network_mesh, fem_mesh, pred, trace=False):
    """Compile+run on 8 cores; returns (loss_scalar, BassKernelResults)."""
    global _PROGRAM
    from concourse.bass_utils import run_bass_kernel_spmd

    _patch_compile_hook()
    if _PROGRAM is None:
        _PROGRAM = _build_program()
    in_maps = _shard_inputs(network_mesh, fem_mesh, pred)
    res = run_bass_kernel_spmd(_PROGRAM, in_maps, list(range(N_CORES)), trace=trace)
    fem_sum = 0.0
    reg_sum = 0.0
    for c in range(N_CORES):
        o = np.asarray(res.results[c]["out"], dtype=np.float64)
        fem_sum += o[:, 0].sum()
        reg_sum += o[:, 1].sum()
    loss = fem_sum / FEM_TOTAL + 0.1 * (reg_sum / (B * C))
    return np.asarray(loss, dtype=np.float32), res


def kernel(network_mesh, pc, fem_mesh, pred):
    loss, _ = run_sharded(network_mesh, fem_mesh, pred, trace=False)
    return loss
